# revision 10
# baseline (speedup 1.0000x reference)
"""Trainium2 Bass kernel for nn_Blobber (3x3 box conv + steep sigmoid, x2).

The reference iterates 4 times but re-convolves the ORIGINAL input each
iteration, so all iterations are identical: the computation collapses to
    y = sigmoid((box3x3(sigmoid((box3x3(x) - 0.01*9) * 1000/9)) - 0.9*9) * 1000/9)
i.e. conv -> sigmoid -> conv -> sigmoid, once.

Implementation (per core, pure data-parallel over batch):
  Each separable 3-tap pass is a TensorE matmul with the image chunk as the
  stationary operand and a narrow banded (tridiagonal) matrix as the moving
  operand.  out[m,n] = sum_k lhsT[k,m] rhs[k,n] contracts the partition dim
  and transposes the layout, so alternating stages apply the vertical /
  horizontal passes with no explicit transposes and no halo exchange; the
  2-column band overlaps between contraction chunks accumulate in PSUM via
  the per-element has_written bits (first chunk start=True, rest accumulate).

  v2 schedule: per-image chains emitted as a skewed software pipeline
  (wavefront) so every engine's program-order stream is dense and each
  image's output stores right after its sig2 — overlapping the remaining
  images' input loads on the other DMA ring.  bf16 intermediates, PSUM f32.
"""

import sys

for _p in ("/opt/trn_rl_repo",):
    if _p not in sys.path:
        sys.path.append(_p)

import numpy as np
import ml_dtypes

import concourse.bass as bass
import concourse.mybir as mybir
from concourse import bacc
from concourse.tile import TileContext
from concourse.bass_utils import run_bass_kernel_spmd

N_CORES = 8
B = 32
H = W = 512
P = 128
NT = H // P                # 4 row-chunks per image
FREE = NT * W              # 2048
IMGS = B // N_CORES        # 4 images per core
SCALE = 1000.0 / 9.0       # folds the 1/9 box normalization into the sigmoid
BIAS1 = -0.01 * 1000.0     # sigmoid((s/9 - 0.01)*1000) = sigmoid(s*SCALE - 10)
BIAS2 = -0.9 * 1000.0

_BF16 = mybir.dt.bfloat16
_F32 = mybir.dt.float32

# --- build config (A/B knobs) -------------------------------------------
CFG = dict(
    load="sw",       # "sw": SWDGE cast f32->bf16 loads; "hw": HWDGE f32 + on-chip cast
    caster="gpsimd", # engine for the on-chip cast when load=="hw"
    skew=4,          # fine-stage skew between consecutive images in the wavefront
    warmup=48,       # scratch matmuls to flip the PE HAM clock gate early
    halves=2,        # how many leading images get half-split loads
)


def _band_matrix() -> np.ndarray:
    """T[k, j] = 1 iff j in {k, k+1, k+2}; moving operand of every stage.

    rhs column j of contraction-chunk t maps to output position 128*t - 1 + j,
    so out gets taps from inputs 128*t+k with |out - in| <= 1.
    """
    t = np.zeros((P, 130), np.float32)
    k = np.arange(P)
    for d in range(3):
        t[k, k + d] = 1.0
    return t.astype(ml_dtypes.bfloat16)


def _bias_matrix() -> np.ndarray:
    """Per-partition bias columns for the two sigmoids (f32)."""
    b = np.empty((P, 2), np.float32)
    b[:, 0] = BIAS1
    b[:, 1] = BIAS2
    return b


def _emit_stage_half(nc, pt, src, tb, half, t_outer=False):
    """Half of one separable 3-tap pass: the two output banks c in
    {2*half, 2*half+1}, full contraction over src's partition chunks.

    src: SBUF bf16 [128, 2048], layout [d1-local-partition, (d1-chunk, d2)]
    pt:  PSUM f32 tile [128, 1024] (2 banks) in the transposed layout
         [d2-local-partition, (d2-chunk - 2*half, d1)]
    t_outer lets stage A start while only the first d1-half is loaded.
    """
    cs = (2 * half, 2 * half + 1)
    pairs = (
        [(t, c) for t in range(NT) for c in cs]
        if t_outer
        else [(t, c) for c in cs for t in range(NT)]
    )
    for t, c in pairs:
        j0 = 1 if t == 0 else 0
        j1 = 129 if t == NT - 1 else 130
        h0 = 128 * t - 1 + j0
        h1 = 128 * t - 1 + j1
        rhs = tb[:, j0:j1]
        lhsT = src[:, t * W + 128 * c : t * W + 128 * c + 128]
        out = pt[:, (c % 2) * W + h0 : (c % 2) * W + h1]
        nc.tensor.matmul(out, lhsT, rhs, start=(t == 0), stop=(t == NT - 1))


def _build_bass(cfg=None):
    cfg = dict(CFG, **(cfg or {}))
    psz = FREE // 2            # half-image: 2 PSUM banks, 1024 free
    sw_load = cfg["load"] == "sw"
    nc = bacc.Bacc("TRN2", target_bir_lowering=False, debug=False)
    x = nc.dram_tensor("x", [IMGS * H, W], _F32, kind="ExternalInput")
    tband = nc.dram_tensor("tband", [P, 130], _BF16, kind="ExternalInput")
    tbias = nc.dram_tensor("tbias", [P, 2], _F32, kind="ExternalInput")
    y = nc.dram_tensor("y", [IMGS * H, W], _F32, kind="ExternalOutput")

    with TileContext(nc) as tc:
        with (
            tc.tile_pool(name="const", bufs=1) as cpool,
            tc.tile_pool(name="xin", bufs=1) as xpool,
            tc.tile_pool(name="xbf", bufs=2) as xbpool,
            tc.tile_pool(name="mid", bufs=2) as p1pool,
            tc.tile_pool(name="sig", bufs=2) as s1pool,
            tc.tile_pool(name="mid2", bufs=2) as p2pool,
            tc.tile_pool(name="outp", bufs=1) as opool,
            tc.tile_pool(name="psum", bufs=4, space="PSUM") as pspool,
        ):
            sig = mybir.ActivationFunctionType.Sigmoid

            # Input loads are the very first instructions.  sw: SWDGE casts
            # f32->bf16 during the DMA (no engine cost, Q0).  hw: HWDGE raw
            # f32 on the sync ring (RTL descriptor gen, earlier first byte),
            # cast on-chip.  Leading images split in d1-halves so stage A
            # (t-outer) starts on the first half.
            xts = []
            for i in range(IMGS):
                dt = _BF16 if sw_load else _F32
                xt = xpool.tile([P, FREE], dt, tag=f"x{i}", name=f"x_{i}")
                halves = 2 if i < cfg["halves"] else 1
                step = NT // halves
                for hh in range(halves):
                    src = x[
                        (i * NT + hh * step) * P : (i * NT + (hh + 1) * step) * P, :
                    ].rearrange("(t p) w -> p t w", p=P)
                    dst = xt[:, hh * step * W : (hh + 1) * step * W].rearrange(
                        "p (t w) -> p t w", t=step
                    )
                    if sw_load:
                        nc.gpsimd.dma_start(out=dst, in_=src)
                    else:
                        # scalar (ACT) HWDGE ring: dispatched before any
                        # ACTIVATE is queued, keeps the sync ring free for
                        # the output stores
                        nc.scalar.dma_start(out=dst, in_=src)
                xts.append(xt)

            # constants on the sync HWDGE ring (tiny; ahead of any stores)
            tb = cpool.tile([P, 130], _BF16)
            nc.sync.dma_start(out=tb[:], in_=tband[:, :])
            bias = cpool.tile([P, 2], _F32, tag="bias")
            nc.sync.dma_start(out=bias[:], in_=tbias[:, :])
            bias1, bias2 = bias[:, 0:1], bias[:, 1:2]

            if cfg["warmup"]:
                # HAM warm-up: scratch matmuls while the input DMAs stream;
                # flips the PE clock gate to 8/8 (2.4 GHz) before the first
                # real stage.
                wsrc = cpool.tile([P, 256], _BF16, tag="wsrc")
                nc.vector.memset(wsrc[:], 0.0)
                wps = pspool.tile([P, psz], _F32, tag="ps", name="wps")
                for _ in range(cfg["warmup"]):
                    nc.tensor.matmul(
                        wps[:, 0:256], wsrc[:, 0:128], wsrc[:, 0:256],
                        start=True, stop=True,
                    )

            caster = getattr(nc, cfg["caster"])

            def make_chain(i):
                """16 fine ops (half-image granularity), each a closure.

                Per half h:  A.h  -> cp1.h (DVE) -> B.h -> sig1.h (ACT)
                          -> C.h -> cp2.h (DVE) -> D.h -> sig2.h+store.
                Stage banks are 2-bank PSUM tiles, so 4 half-stages are in
                flight at once (full 8-bank PSUM) and drains free banks at
                half-image granularity.
                """
                st = {}
                if sw_load:
                    st["xb"] = xts[i]

                def ps_tile(nm):
                    return pspool.tile([P, psz], _F32, tag="ps", name=f"{nm}_{i}")

                def s_cast(h):
                    def f():
                        if h == 0:
                            st["xb"] = xbpool.tile(
                                [P, FREE], _BF16, tag="xb", name=f"xb{i}"
                            )
                        sl = slice(h * psz, (h + 1) * psz)
                        caster.tensor_copy(st["xb"][:, sl], xts[i][:, sl])
                    return f

                def s_mm(nm, src_key, dst_key, h, t_outer=False):
                    def f():
                        if h == 0:
                            st[dst_key] = [None, None]
                        pt = ps_tile(f"{nm}{h}")
                        st[dst_key][h] = pt
                        _emit_stage_half(
                            nc, pt, st[src_key], tb, h, t_outer=t_outer
                        )
                    return f

                def s_copy(src_key, dst_key, pool, h):
                    def f():
                        if h == 0:
                            st[dst_key] = pool.tile(
                                [P, FREE], _BF16, tag=dst_key, name=f"{dst_key}{i}"
                            )
                        nc.vector.tensor_copy(
                            st[dst_key][:, h * psz : (h + 1) * psz],
                            st[src_key][h][:],
                        )
                    return f

                def s_sig1(h):
                    def f():
                        if h == 0:
                            st["s1"] = s1pool.tile(
                                [P, FREE], _BF16, tag="s1", name=f"s1_{i}"
                            )
                        nc.scalar.activation(
                            st["s1"][:, h * psz : (h + 1) * psz],
                            st["pb"][h][:],
                            sig,
                            bias=bias1,
                            scale=SCALE,
                        )
                    return f

                def s_sig2(h):
                    def f():
                        if h == 0:
                            st["ot"] = opool.tile(
                                [P, FREE], _F32, tag=f"o{i}", name=f"o_{i}"
                            )
                        sl = slice(h * psz, (h + 1) * psz)
                        nc.scalar.activation(
                            st["ot"][:, sl], st["pd"][h][:], sig, bias=bias2,
                            scale=SCALE,
                        )
                        rows = NT // 2 * P
                        nc.sync.dma_start(
                            out=y[
                                i * H + h * rows : i * H + (h + 1) * rows, :
                            ].rearrange("(t p) w -> p t w", p=P),
                            in_=st["ot"][:, sl].rearrange(
                                "p (t w) -> p t w", t=NT // 2
                            ),
                        )
                    return f

                ops = []
                if not sw_load:
                    ops += [s_cast(0), s_cast(1)]
                for h in (0, 1):
                    ops.append(s_mm("pa", "xb", "pa", h, t_outer=True))
                for h in (0, 1):
                    ops.append(s_copy("pa", "p1", p1pool, h))
                for h in (0, 1):
                    ops.append(s_mm("pb", "p1", "pb", h))
                ops += [s_sig1(0), s_sig1(1)]
                for h in (0, 1):
                    ops.append(s_mm("pc", "s1", "pc", h))
                for h in (0, 1):
                    ops.append(s_copy("pc", "p2", p2pool, h))
                for h in (0, 1):
                    ops.append(s_mm("pd", "p2", "pd", h))
                ops += [s_sig2(0), s_sig2(1)]
                return ops

            chains = [make_chain(i) for i in range(IMGS)]
            nstg = len(chains[0])
            skew = cfg["skew"]
            for w in range(skew * (IMGS - 1) + nstg):
                for i in range(IMGS):
                    s = w - skew * i
                    if 0 <= s < nstg:
                        chains[i][s]()
    nc.compile()
    return nc


_NC_CACHE = {}


def _get_nc(key=None):
    if key not in _NC_CACHE:
        _NC_CACHE[key] = _build_bass()
    return _NC_CACHE[key]


def kernel_with_results(inputs: np.ndarray, **run_kwargs):
    """inputs: [32, 1, 512, 512] f32. Returns (out [32,1,512,512] f32, results)."""
    x = np.asarray(inputs)
    assert x.shape == (B, 1, H, W), x.shape
    x = np.ascontiguousarray(x.reshape(B, H, W), dtype=np.float32)
    tb = np.ascontiguousarray(_band_matrix())
    tbias = np.ascontiguousarray(_bias_matrix())

    in_maps = []
    for k in range(N_CORES):
        xk = np.ascontiguousarray(
            x[k * IMGS : (k + 1) * IMGS].reshape(IMGS * H, W)
        )
        in_maps.append({"x": xk, "tband": tb, "tbias": tbias})

    nc = _get_nc()
    res = run_bass_kernel_spmd(nc, in_maps, core_ids=list(range(N_CORES)), **run_kwargs)
    out = np.empty((B, H, W), dtype=np.float32)
    for k in range(N_CORES):
        out[k * IMGS : (k + 1) * IMGS] = (
            np.asarray(res.results[k]["y"]).astype(np.float32).reshape(IMGS, H, W)
        )
    return out.reshape(B, 1, H, W), res


def kernel(inputs: np.ndarray) -> np.ndarray:
    out, _ = kernel_with_results(inputs)
    return out


if __name__ == "__main__":
    rng = np.random.default_rng(0)
    demo = rng.random((B, 1, H, W), dtype=np.float32)
    out = kernel(demo)
    print("out", out.shape, out.dtype, float(out.min()), float(out.max()))


# revision 14
# speedup vs baseline: 1.4564x; 1.4564x over previous
"""Trainium2 Bass kernel for nn_Blobber (3x3 box conv + steep sigmoid, x2).

The reference iterates 4 times but re-convolves the ORIGINAL input each
iteration, so all iterations are identical: the computation collapses to
    y = sigmoid((box3x3(sigmoid((box3x3(x) - 0.01*9) * 1000/9)) - 0.9*9) * 1000/9)
i.e. conv -> sigmoid -> conv -> sigmoid, once.

Implementation (per core, pure data-parallel over batch):
  Each separable 3-tap pass is a TensorE matmul with the image chunk as the
  stationary operand and a narrow banded (tridiagonal) matrix as the moving
  operand.  out[m,n] = sum_k lhsT[k,m] rhs[k,n] contracts the partition dim
  and transposes the layout, so alternating stages apply the vertical /
  horizontal passes with no explicit transposes and no halo exchange; the
  2-column band overlaps between contraction chunks accumulate in PSUM via
  the per-element has_written bits (first chunk start=True, rest accumulate).

  v2 schedule: per-image chains emitted as a skewed software pipeline
  (wavefront) so every engine's program-order stream is dense and each
  image's output stores right after its sig2 — overlapping the remaining
  images' input loads on the other DMA ring.  bf16 intermediates, PSUM f32.
"""

import sys

for _p in ("/opt/trn_rl_repo",):
    if _p not in sys.path:
        sys.path.append(_p)

import numpy as np
import ml_dtypes

import concourse.bass as bass
import concourse.mybir as mybir
from concourse import bacc
from concourse.tile import TileContext
from concourse.bass_utils import run_bass_kernel_spmd

N_CORES = 8
B = 32
H = W = 512
P = 128
NT = H // P                # 4 row-chunks per image
FREE = NT * W              # 2048
IMGS = B // N_CORES        # 4 images per core
SCALE = 1000.0 / 9.0       # folds the 1/9 box normalization into the sigmoid
BIAS1 = -0.01 * 1000.0     # sigmoid((s/9 - 0.01)*1000) = sigmoid(s*SCALE - 10)
BIAS2 = -0.9 * 1000.0

_BF16 = mybir.dt.bfloat16
_F32 = mybir.dt.float32

# --- build config (A/B knobs) -------------------------------------------
CFG = dict(
    load="sw",       # "sw": SWDGE cast f32->bf16 loads; "hw": HWDGE f32 + on-chip cast
    caster="gpsimd", # engine for the on-chip cast when load=="hw"
    skew=4,          # fine-stage skew between consecutive images in the wavefront
    warmup=24,       # scratch matmuls to flip the PE HAM clock gate early
    halves=0,        # how many leading images get half-split loads
)


def _band_matrix() -> np.ndarray:
    """T[k, j] = 1 iff j in {k, k+1, k+2}; moving operand of every stage.

    rhs column j of contraction-chunk t maps to output position 128*t - 1 + j,
    so out gets taps from inputs 128*t+k with |out - in| <= 1.
    """
    t = np.zeros((P, 130), np.float32)
    k = np.arange(P)
    for d in range(3):
        t[k, k + d] = 1.0
    return t.astype(ml_dtypes.bfloat16)


def _bias_matrix() -> np.ndarray:
    """Per-partition bias columns for the two sigmoids (f32)."""
    b = np.empty((P, 2), np.float32)
    b[:, 0] = BIAS1
    b[:, 1] = BIAS2
    return b


def _emit_stage_half(nc, pt, src, tb, half, t_outer=False):
    """Half of one separable 3-tap pass: the two output banks c in
    {2*half, 2*half+1}, full contraction over src's partition chunks.

    src: SBUF bf16 [128, 2048], layout [d1-local-partition, (d1-chunk, d2)]
    pt:  PSUM f32 tile [128, 1024] (2 banks) in the transposed layout
         [d2-local-partition, (d2-chunk - 2*half, d1)]
    t_outer lets stage A start while only the first d1-half is loaded.
    """
    cs = (2 * half, 2 * half + 1)
    pairs = (
        [(t, c) for t in range(NT) for c in cs]
        if t_outer
        else [(t, c) for c in cs for t in range(NT)]
    )
    for t, c in pairs:
        j0 = 1 if t == 0 else 0
        j1 = 129 if t == NT - 1 else 130
        h0 = 128 * t - 1 + j0
        h1 = 128 * t - 1 + j1
        rhs = tb[:, j0:j1]
        lhsT = src[:, t * W + 128 * c : t * W + 128 * c + 128]
        out = pt[:, (c % 2) * W + h0 : (c % 2) * W + h1]
        nc.tensor.matmul(out, lhsT, rhs, start=(t == 0), stop=(t == NT - 1))


def _build_bass(cfg=None):
    cfg = dict(CFG, **(cfg or {}))
    psz = FREE // 2            # half-image: 2 PSUM banks, 1024 free
    sw_load = cfg["load"] == "sw"
    nc = bacc.Bacc("TRN2", target_bir_lowering=False, debug=False)
    x = nc.dram_tensor("x", [IMGS * H, W], _F32, kind="ExternalInput")
    tband = nc.dram_tensor("tband", [P, 130], _BF16, kind="ExternalInput")
    tbias = nc.dram_tensor("tbias", [P, 2], _F32, kind="ExternalInput")
    y = nc.dram_tensor("y", [IMGS * H, W], _F32, kind="ExternalOutput")

    with TileContext(nc) as tc:
        with (
            tc.tile_pool(name="const", bufs=1) as cpool,
            tc.tile_pool(name="xin", bufs=1) as xpool,
            tc.tile_pool(name="xbf", bufs=2) as xbpool,
            tc.tile_pool(name="mid", bufs=2) as p1pool,
            tc.tile_pool(name="sig", bufs=2) as s1pool,
            tc.tile_pool(name="mid2", bufs=2) as p2pool,
            tc.tile_pool(name="outp", bufs=1) as opool,
            tc.tile_pool(name="psum", bufs=4, space="PSUM") as pspool,
        ):
            sig = mybir.ActivationFunctionType.Sigmoid

            # Warm-up scratch init first: a 115ns gpsimd memset ahead of the
            # load dispatches lets the HAM warm-up matmuls start right after
            # the preamble instead of waiting on another engine's stream.
            wsrc = None
            if cfg["warmup"]:
                wsrc = cpool.tile([P, 256], _BF16, tag="wsrc")
                nc.gpsimd.memset(wsrc[:], 0.0)

            # Input loads are the very first instructions.  sw: SWDGE casts
            # f32->bf16 during the DMA (no engine cost, Q0).  hw: HWDGE raw
            # f32 on the sync ring (RTL descriptor gen, earlier first byte),
            # cast on-chip.  Leading images split in d1-halves so stage A
            # (t-outer) starts on the first half.
            xts = []
            for i in range(IMGS):
                dt = _BF16 if sw_load else _F32
                xt = xpool.tile([P, FREE], dt, tag=f"x{i}", name=f"x_{i}")
                halves = 2 if i < cfg["halves"] else 1
                step = NT // halves
                for hh in range(halves):
                    src = x[
                        (i * NT + hh * step) * P : (i * NT + (hh + 1) * step) * P, :
                    ].rearrange("(t p) w -> p t w", p=P)
                    dst = xt[:, hh * step * W : (hh + 1) * step * W].rearrange(
                        "p (t w) -> p t w", t=step
                    )
                    if sw_load:
                        nc.gpsimd.dma_start(out=dst, in_=src)
                    else:
                        # scalar (ACT) HWDGE ring: dispatched before any
                        # ACTIVATE is queued, keeps the sync ring free for
                        # the output stores
                        nc.scalar.dma_start(out=dst, in_=src)
                xts.append(xt)

            # constants on the sync HWDGE ring (tiny; ahead of any stores)
            tb = cpool.tile([P, 130], _BF16)
            nc.sync.dma_start(out=tb[:], in_=tband[:, :])
            bias = cpool.tile([P, 2], _F32, tag="bias")
            nc.sync.dma_start(out=bias[:], in_=tbias[:, :])
            bias1, bias2 = bias[:, 0:1], bias[:, 1:2]

            if cfg["warmup"]:
                # HAM warm-up: scratch matmuls while the input DMAs stream;
                # flips the PE clock gate to 8/8 (2.4 GHz) before the first
                # real stage.
                wps = pspool.tile([P, psz], _F32, tag="ps", name="wps")
                for _ in range(cfg["warmup"]):
                    nc.tensor.matmul(
                        wps[:, 0:256], wsrc[:, 0:128], wsrc[:, 0:256],
                        start=True, stop=True,
                    )

            caster = getattr(nc, cfg["caster"])

            def make_chain(i):
                """16 fine ops (half-image granularity), each a closure.

                Per half h:  A.h  -> cp1.h (DVE) -> B.h -> sig1.h (ACT)
                          -> C.h -> cp2.h (DVE) -> D.h -> sig2.h+store.
                Stage banks are 2-bank PSUM tiles, so 4 half-stages are in
                flight at once (full 8-bank PSUM) and drains free banks at
                half-image granularity.
                """
                st = {}
                if sw_load:
                    st["xb"] = xts[i]

                def ps_tile(nm):
                    return pspool.tile([P, psz], _F32, tag="ps", name=f"{nm}_{i}")

                def s_cast(h):
                    def f():
                        if h == 0:
                            st["xb"] = xbpool.tile(
                                [P, FREE], _BF16, tag="xb", name=f"xb{i}"
                            )
                        sl = slice(h * psz, (h + 1) * psz)
                        caster.tensor_copy(st["xb"][:, sl], xts[i][:, sl])
                    return f

                def s_mm(nm, src_key, dst_key, h, t_outer=False):
                    def f():
                        if h == 0:
                            st[dst_key] = [None, None]
                        pt = ps_tile(f"{nm}{h}")
                        st[dst_key][h] = pt
                        _emit_stage_half(
                            nc, pt, st[src_key], tb, h, t_outer=t_outer
                        )
                    return f

                def s_copy(src_key, dst_key, pool, h):
                    def f():
                        if h == 0:
                            st[dst_key] = pool.tile(
                                [P, FREE], _BF16, tag=dst_key, name=f"{dst_key}{i}"
                            )
                        nc.vector.tensor_copy(
                            st[dst_key][:, h * psz : (h + 1) * psz],
                            st[src_key][h][:],
                        )
                    return f

                def s_sig1(h):
                    def f():
                        if h == 0:
                            st["s1"] = s1pool.tile(
                                [P, FREE], _BF16, tag="s1", name=f"s1_{i}"
                            )
                        nc.scalar.activation(
                            st["s1"][:, h * psz : (h + 1) * psz],
                            st["pb"][h][:],
                            sig,
                            bias=bias1,
                            scale=SCALE,
                        )
                    return f

                def s_sig2(h):
                    def f():
                        if h == 0:
                            st["ot"] = opool.tile(
                                [P, FREE], _F32, tag=f"o{i}", name=f"o_{i}"
                            )
                        sl = slice(h * psz, (h + 1) * psz)
                        nc.scalar.activation(
                            st["ot"][:, sl], st["pd"][h][:], sig, bias=bias2,
                            scale=SCALE,
                        )
                        rows = NT // 2 * P
                        nc.sync.dma_start(
                            out=y[
                                i * H + h * rows : i * H + (h + 1) * rows, :
                            ].rearrange("(t p) w -> p t w", p=P),
                            in_=st["ot"][:, sl].rearrange(
                                "p (t w) -> p t w", t=NT // 2
                            ),
                        )
                    return f

                ops = []
                if not sw_load:
                    ops += [s_cast(0), s_cast(1)]
                for h in (0, 1):
                    ops.append(s_mm("pa", "xb", "pa", h, t_outer=True))
                for h in (0, 1):
                    ops.append(s_copy("pa", "p1", p1pool, h))
                for h in (0, 1):
                    ops.append(s_mm("pb", "p1", "pb", h))
                ops += [s_sig1(0), s_sig1(1)]
                for h in (0, 1):
                    ops.append(s_mm("pc", "s1", "pc", h))
                for h in (0, 1):
                    ops.append(s_copy("pc", "p2", p2pool, h))
                for h in (0, 1):
                    ops.append(s_mm("pd", "p2", "pd", h))
                ops += [s_sig2(0), s_sig2(1)]
                return ops

            chains = [make_chain(i) for i in range(IMGS)]
            nstg = len(chains[0])
            skew = cfg["skew"]
            for w in range(skew * (IMGS - 1) + nstg):
                for i in range(IMGS):
                    s = w - skew * i
                    if 0 <= s < nstg:
                        chains[i][s]()
    nc.compile()
    return nc


_NC_CACHE = {}


def _get_nc(key=None):
    if key not in _NC_CACHE:
        _NC_CACHE[key] = _build_bass()
    return _NC_CACHE[key]


def kernel_with_results(inputs: np.ndarray, **run_kwargs):
    """inputs: [32, 1, 512, 512] f32. Returns (out [32,1,512,512] f32, results)."""
    x = np.asarray(inputs)
    assert x.shape == (B, 1, H, W), x.shape
    x = np.ascontiguousarray(x.reshape(B, H, W), dtype=np.float32)
    tb = np.ascontiguousarray(_band_matrix())
    tbias = np.ascontiguousarray(_bias_matrix())

    in_maps = []
    for k in range(N_CORES):
        xk = np.ascontiguousarray(
            x[k * IMGS : (k + 1) * IMGS].reshape(IMGS * H, W)
        )
        in_maps.append({"x": xk, "tband": tb, "tbias": tbias})

    nc = _get_nc()
    res = run_bass_kernel_spmd(nc, in_maps, core_ids=list(range(N_CORES)), **run_kwargs)
    out = np.empty((B, H, W), dtype=np.float32)
    for k in range(N_CORES):
        out[k * IMGS : (k + 1) * IMGS] = (
            np.asarray(res.results[k]["y"]).astype(np.float32).reshape(IMGS, H, W)
        )
    return out.reshape(B, 1, H, W), res


def kernel(inputs: np.ndarray) -> np.ndarray:
    out, _ = kernel_with_results(inputs)
    return out


if __name__ == "__main__":
    rng = np.random.default_rng(0)
    demo = rng.random((B, 1, H, W), dtype=np.float32)
    out = kernel(demo)
    print("out", out.shape, out.dtype, float(out.min()), float(out.max()))
